# revision 1
# baseline (speedup 1.0000x reference)
"""GNN message-passing aggregation kernel for Trainium2 (8 NeuronCores).

Math: y[n,o] = mean_k relu(mailbox[n,k,:] @ W1 + b1) @ W2 + b2
  mailbox [500000, 16, 7] fp32, W1 [7,40], W2 [40,3], b1 == 0 (asserted).

Host prep: X^T [112, nodes] bf16 per core (transpose + cast on host), so
the per-tile X^T [112,128] slice is the PE stationary directly -- no
on-device transpose, no dtype cast.

Per 128-node tile, z = X W1blk (640 cols) lands in PSUM via 2 matmuls.
Drain path alternates per tile (pattern PATHS):
  'd'  DVE fused abs-reduce (sum_k relu z = (sum z + sum |z|)/2; the
       sum-z term goes to y directly via a 3-col PE matmul with
       tile_k(W1 @ W2)/32 weights)
  'v'  ACT relu-drain to bf16 + DVE k-reduce from SBUF
  'g2' ACT relu-drain (pair-permuted weight cols) + one GPSIMD halving
       add + short DVE k-reduce
Per tile-pair the two 40-col hsums go through one DMA transpose into a
[128,128] stationary, then ONE 6-col matmul applies the zero-masked W2
stack (W2/32 rows 0:40 for the abs half, W2/16 rows 64:104 for the relu
half) accumulating y into a 160-tile PSUM group; groups flush via DVE
add (+b2) and a chunked DMA out.

Sharding: pure data parallel over nodes, 62500/core (padded 62592).
"""

import os
import numpy as np
import ml_dtypes
from contextlib import ExitStack

import concourse.bass as bass
import concourse.bacc as bacc
import concourse.tile as tile
import concourse.mybir as mybir
from concourse.bass import ds, ts
from concourse import bass_utils

F32 = mybir.dt.float32
BF16 = mybir.dt.bfloat16

N_FULL = 500000
K, F_IN, F_HID, F_OUT = 16, 7, 40, 3
FB = K * F_IN              # 112
N_CORES = 8
TILE_P = 128
NODES_REAL_PER_CORE = N_FULL // N_CORES          # 62500
CHUNK = 16                 # node-tiles per input DMA
GROUP = 160                # tiles per y-psum group (160*3 = 480 psum cols)

# drain-path pattern over tile index i % len(PATHS); even slots must be
# 'd' (pair half A, W2/32 + direct), odd slots relu-type ('v' or 'g2',
# half B, W2/16).
PATHS = ['d', 'g2']

# fraction of hsum transposes issued from the scalar (ACT) HWDGE queue
# instead of sync; pairs with (pair_idx % TSPLIT_MOD) < TSPLIT_NUM go to
# scalar.
TSPLIT_NUM, TSPLIT_MOD = 0, 4

# L2 matmuls are emitted LAG pairs behind their transpose so the PE
# FIFO never head-of-line blocks on an in-flight DMA transpose.
LAG = 4

HB_BUFS = 6

# w1b column layout: [0:640) k-inner z cols (col 16j+k), [640:643) direct
# (W1 @ W2)/32 cols, [643:1283) g2-permuted z cols (8j+k for k<8 at
# +643, 320+8j+k-8 for k>=8).
W1B_COLS = 1283


def build(nc, n_tiles, level=4):
    """Emit the full per-core program into nc.

    level: ablation. 4 = full; 3 = no L2/transpose (y = b2 only);
    2 = L1 + drains, no hb consumers; 1 = L1 matmuls only; 0 = DMA only.
    """
    n_nodes = n_tiles * TILE_P
    x = nc.dram_tensor("x", (FB, n_nodes), BF16, kind="ExternalInput")
    w1b = nc.dram_tensor("w1b", (FB, W1B_COLS), BF16, kind="ExternalInput")
    w2b = nc.dram_tensor("w2b", (TILE_P, 2 * F_OUT), BF16, kind="ExternalInput")
    b2rep = nc.dram_tensor("b2rep", (TILE_P, 3 * GROUP), F32, kind="ExternalInput")
    y = nc.dram_tensor("y", (n_nodes, F_OUT), F32, kind="ExternalOutput")

    xap = x.ap()
    yap = y.ap()
    n_pat = len(PATHS)

    with ExitStack() as ctx:
        tc = ctx.enter_context(tile.TileContext(nc))
        const = ctx.enter_context(tc.tile_pool(name="const", bufs=1))
        xinp = ctx.enter_context(tc.tile_pool(name="xin", bufs=4))
        zp = ctx.enter_context(tc.tile_pool(name="z", bufs=3, space="PSUM"))
        habsp = ctx.enter_context(tc.tile_pool(name="habs", bufs=4))
        treep = ctx.enter_context(tc.tile_pool(name="tree", bufs=4))
        htp = ctx.enter_context(tc.tile_pool(name="ht", bufs=LAG + 2))
        ypsp = ctx.enter_context(tc.tile_pool(name="yps", bufs=2, space="PSUM"))
        ysbp = ctx.enter_context(tc.tile_pool(name="ysb", bufs=2))

        w1b_sb = const.tile([FB, W1B_COLS], BF16)
        nc.sync.dma_start(w1b_sb[:], w1b.ap())
        w2b_sb = const.tile([TILE_P, 2 * F_OUT], BF16)
        nc.sync.dma_start(w2b_sb[:], w2b.ap())
        b2rep_sb = const.tile([TILE_P, 3 * GROUP], F32)
        nc.sync.dma_start(b2rep_sb[:], b2rep.ap())
        zconst = const.tile([FB, 128], BF16)
        nc.gpsimd.memset(zconst[:], 0.0)

        # fixed hb buffers (stable tensor ids): cols 40:64, 104:128 are
        # zeroed once and never rewritten; they feed zero rows of w2b
        # after transpose but must be finite, not garbage.
        hb_bufs = []
        for bi in range(HB_BUFS):
            hb0 = const.tile([TILE_P, 128], BF16, tag=f"hb{bi}")
            nc.gpsimd.memset(hb0[:, 43:64], 0.0)
            nc.gpsimd.memset(hb0[:, 104:128], 0.0)
            hb_bufs.append(hb0)

        xin = hb = yps = None
        yps_by_group = {}
        pending = []

        def pop_pending():
            e = pending.pop(0)
            nc.tensor.matmul(
                e["yps"][:, e["cols"]], e["ht"][:, :], w2b_sb[:, e["wcols"]],
                start=False, stop=False, skip_group_check=True,
            )
            if e["flush"] is not None:
                flush_group(*e["flush"])

        def flush_group(g_yps, g_base, g_ntiles):
            """Drain yps group to SBUF (+b2) and DMA to DRAM."""
            ncols = 3 * g_ntiles
            ysb = ysbp.tile([TILE_P, 3 * GROUP], F32, tag="ysb")
            if g_yps is None:
                nc.vector.tensor_copy(ysb[:, 0:ncols], b2rep_sb[:, 0:ncols])
            else:
                # close the bank's accumulation group (adds zero, full-bank
                # WAW orders it after every per-tile accumulate).
                nc.tensor.matmul(
                    g_yps[:, 0 : 3 * GROUP], zconst[:],
                    w1b_sb[:, 0 : 3 * GROUP],
                    start=False, stop=True, skip_group_check=True,
                )
                nc.vector.tensor_add(
                    ysb[:, 0:ncols], g_yps[:, 0:ncols], b2rep_sb[:, 0:ncols]
                )
            n_full_chunks = g_ntiles // CHUNK
            if n_full_chunks:
                nn = n_full_chunks * CHUNK * TILE_P
                dst = yap[ds(g_base * TILE_P, nn), :].rearrange(
                    "(c q s) o -> q c s o", q=TILE_P, s=CHUNK
                )
                src_ap = ysb[:, 0 : n_full_chunks * CHUNK * 3].rearrange(
                    "q (c s o) -> q c s o", s=CHUNK, o=3
                )
                nc.sync.dma_start(dst, src_ap)
            rem = g_ntiles - n_full_chunks * CHUNK
            if rem:
                base = g_base + n_full_chunks * CHUNK
                dst = yap[ds(base * TILE_P, rem * TILE_P), :].rearrange(
                    "(q s) o -> q s o", s=rem
                )
                src_ap = ysb[
                    :, n_full_chunks * CHUNK * 3 : g_ntiles * 3
                ].rearrange("q (s o) -> q s o", o=3)
                nc.sync.dma_start(dst, src_ap)

        # Software-pipelined emission: per-engine queues are strict
        # program order, so each tile's dependent stages are emitted N
        # rounds later to keep every queue head runnable.
        zabs, habss, trs = {}, {}, {}
        xins = {}
        n_chunks = (n_tiles + CHUNK - 1) // CHUNK
        PREFETCH = 2

        def emit_chunk_dma(c):
            if c >= n_chunks or c in xins:
                return
            nch = min(CHUNK, n_tiles - c * CHUNK)
            xin = xinp.tile([FB, CHUNK * TILE_P], BF16, tag="xin")
            # scalar HWDGE queue: keeps input DMAs from queueing behind
            # hsum transposes on sync.
            nc.scalar.dma_start(
                xin[:, 0 : nch * TILE_P],
                xap[:, ds(c * CHUNK * TILE_P, nch * TILE_P)],
            )
            xins[c] = xin

        def stage_front(i):
            c, s = divmod(i, CHUNK)
            if i == 0:
                for pc in range(PREFETCH + 1):
                    emit_chunk_dma(pc)
            elif s == 0:
                emit_chunk_dma(c + PREFETCH)
                xins.pop(c - 2, None)
            g_idx = i % GROUP
            if g_idx == 0 and level >= 4:
                yps = ypsp.tile([TILE_P, 3 * GROUP], F32, tag="yps")
                yps_by_group[i // GROUP] = yps
                # open the bank's single accumulation group (whole region
                # pending-zero; orders before every per-tile accumulate).
                nc.tensor.matmul(
                    yps[:, 0 : 3 * GROUP], zconst[:],
                    w1b_sb[:, 0 : 3 * GROUP],
                    start=True, stop=False, skip_group_check=True,
                )
            if level < 1:
                return
            path = PATHS[i % n_pat]
            wofs = 643 if path == 'g2' else 0
            xts = xins[c][:, ds(s * TILE_P, TILE_P)]
            zab = zp.tile([TILE_P, 643], F32, tag="z")
            zabs[i] = zab
            nc.tensor.matmul(
                zab[:, 0:512], xts, w1b_sb[:, wofs : wofs + 512],
                start=True, stop=True,
            )
            # 'd' tiles: cols 640:643 of the second matmul carry the
            # direct term (sum_k z) @ W2/32, routed to y via hb rows
            # 40:43 and identity rows in the W2 stack.
            l1b_w = 131 if path == 'd' else 128
            nc.tensor.matmul(
                zab[:, 512 : 512 + l1b_w],
                xts, w1b_sb[:, wofs + 512 : wofs + 512 + l1b_w],
                start=True, stop=True,
            )
            if path == 'd' and level >= 2:
                # emitted here (lag 0) so it sits AHEAD of the previous
                # g2 relu in the scalar queue: the copy is the last zab
                # reader gating PSUM buffer reuse.
                hb = hb_bufs[(i // 2) % HB_BUFS]
                hcol = 64 * (i % 2)
                with nc.allow_low_precision("bf16 y-direct"):
                    nc.scalar.copy(
                        hb[:, hcol + 40 : hcol + 43], zab[:, 640:643]
                    )

        def stage_drain1(i):
            if level < 2:
                zabs.pop(i, None)
                return
            path = PATHS[i % n_pat]
            zab = zabs.pop(i)
            hb = hb_bufs[(i // 2) % HB_BUFS]
            hcol = 64 * (i % 2)
            with nc.allow_low_precision("bf16 hsum is within tolerance"):
                if path == 'd':
                    nc.vector.tensor_reduce(
                        hb[:, hcol : hcol + 40],
                        zab[:, 0:640].rearrange("q (j k) -> q j k", k=K),
                        axis=mybir.AxisListType.X,
                        op=mybir.AluOpType.add,
                        apply_absolute_value=True,
                    )
                else:
                    habs = habsp.tile([TILE_P, 640], BF16, tag="habs")
                    habss[i] = habs
                    nc.scalar.activation(
                        habs[:], zab[:, 0:640],
                        mybir.ActivationFunctionType.Relu,
                    )

        def stage_fold(i):
            if level < 2 or PATHS[i % n_pat] != 'g2':
                return
            habs = habss.pop(i)
            tr = treep.tile([TILE_P, 320], BF16, tag="tree")
            trs[i] = tr
            nc.gpsimd.tensor_add(
                tr[:, 0:320], habs[:, 0:320], habs[:, 320:640]
            )

        def stage_red2(i):
            if level < 2 or PATHS[i % n_pat] == 'd':
                return
            hb = hb_bufs[(i // 2) % HB_BUFS]
            hcol = 64 * (i % 2)
            with nc.allow_low_precision("bf16 hsum is within tolerance"):
                if PATHS[i % n_pat] == 'g2':
                    tr = trs.pop(i)
                    nc.vector.tensor_reduce(
                        hb[:, hcol : hcol + 40],
                        tr[:, 0:320].rearrange("q (j k) -> q j k", k=8),
                        axis=mybir.AxisListType.X,
                        op=mybir.AluOpType.add,
                    )
                else:  # 'v'
                    habs = habss.pop(i)
                    nc.vector.tensor_reduce(
                        hb[:, hcol : hcol + 40],
                        habs[:].rearrange("q (j k) -> q j k", k=K),
                        axis=mybir.AxisListType.X,
                        op=mybir.AluOpType.add,
                    )

        def stage_xpose(i):
            if level < 4 or not (i % 2 == 1 or i == n_tiles - 1):
                return
            t2 = i % 2
            g_idx = i % GROUP
            pair_idx = i // 2
            hb = hb_bufs[pair_idx % HB_BUFS]
            ht = htp.tile([128, 128], BF16, tag="ht")
            eng = (
                nc.scalar
                if TSPLIT_NUM and (pair_idx % TSPLIT_MOD) < TSPLIT_NUM
                else nc.sync
            )
            eng.dma_start(ht[:], hb[:], transpose=True)
            # one 6-col L2 matmul per pair, emitted LAG pairs later:
            # ht rows 0:43 ([W2/32; I3], tile i-1) -> cols 3(g-1):3g,
            # rows 64:104 (W2/16, tile i) -> cols 3g:3g+3.
            e = {
                "ht": ht,
                "yps": yps_by_group[i // GROUP],
                "cols": ds(3 * (g_idx - 1), 6) if t2 == 1 else ts(g_idx, 3),
                "wcols": ds(0, 6) if t2 == 1 else ds(0, 3),
                "flush": None,
            }
            if g_idx == GROUP - 1 or i == n_tiles - 1:
                e["flush"] = (yps_by_group[i // GROUP], i - g_idx, g_idx + 1)
            pending.append(e)
            if len(pending) > LAG:
                pop_pending()

        for r in range(n_tiles + 5):
            if r < n_tiles:
                stage_front(r)
            if 0 <= r - 1 < n_tiles:
                stage_drain1(r - 1)
            if 0 <= r - 2 < n_tiles:
                stage_fold(r - 2)
            if 0 <= r - 3 < n_tiles:
                stage_red2(r - 3)
            if 0 <= r - 4 < n_tiles:
                stage_xpose(r - 4)

        while pending:
            pop_pending()


_CACHE = {}


def _get_prog():
    key = "prog"
    if key not in _CACHE:
        nc = bacc.Bacc(
            "TRN2", target_bir_lowering=False, debug=False,
            num_devices=N_CORES,
        )
        n_tiles = (NODES_REAL_PER_CORE + TILE_P - 1) // TILE_P  # 489
        build(nc, n_tiles, level=int(os.environ.get("KERNEL_LEVEL", "4")))
        nc.finalize()
        _CACHE[key] = (nc, n_tiles)
    return _CACHE[key]


def _host_weights(W1, b1, W2, b2):
    W1 = np.asarray(W1, np.float32)
    W2 = np.asarray(W2, np.float32)
    b2 = np.asarray(b2, np.float32)

    # k-inner z cols: col 16*j + k
    w1ki = np.zeros((K, F_IN, F_HID, K), np.float32)
    for k in range(K):
        w1ki[k, :, :, k] = W1
    w1ki = w1ki.reshape(FB, F_HID * K)
    # direct term: sum_k z_k @ W2/32 = X @ tile_k(W1 @ W2)/32
    wdir = np.tile(W1 @ W2 / 32.0, (K, 1))  # [112, 3]
    # g2-permuted cols: halves foldable by one contiguous add, result
    # j-major k-inner(8): col 8j+k for k<8, col 320+8j+(k-8) for k>=8.
    w1g2 = np.zeros((FB, 640), np.float32)
    for k in range(K):
        for j in range(F_HID):
            col = 8 * j + k if k < 8 else 320 + 8 * j + (k - 8)
            w1g2[7 * k : 7 * k + 7, col] = W1[:, j]
    w1b = np.concatenate([w1ki, wdir, w1g2], axis=1).astype(ml_dtypes.bfloat16)

    w2rows = np.zeros((TILE_P, 2 * F_OUT), np.float32)
    w2rows[0:F_HID, 0:F_OUT] = W2 / 32.0       # abs half ('d')
    w2rows[F_HID : F_HID + F_OUT, 0:F_OUT] = np.eye(F_OUT)  # direct term
    w2rows[64 : 64 + F_HID, F_OUT : 2 * F_OUT] = W2 / 16.0  # relu half
    w2rows = w2rows.astype(ml_dtypes.bfloat16)
    b2rep = np.tile(b2, (TILE_P, GROUP)).astype(np.float32)
    return w1b, w2rows, b2rep


def kernel(mailbox, W1, b1, W2, b2, **_unused):
    mailbox = np.asarray(mailbox)
    assert mailbox.shape == (N_FULL, K, F_IN), mailbox.shape
    b1 = np.asarray(b1, np.float32)
    assert np.abs(b1).max() == 0.0, "kernel assumes b1 == 0"

    nc, n_tiles = _get_prog()
    n_nodes = n_tiles * TILE_P

    X = np.ascontiguousarray(mailbox, dtype=np.float32).reshape(N_FULL, FB)
    XT = np.ascontiguousarray(X.T.astype(ml_dtypes.bfloat16))  # [112, N]
    w1b, w2rows, b2rep = _host_weights(W1, b1, W2, np.asarray(b2, np.float32))

    # node-interleaved tiling: within each CHUNK-tile block, node
    # base + CHUNK*q + s sits at (tile s, partition q), so the output
    # DMA writes CHUNK*3-element contiguous runs per partition.
    ni = np.empty((n_tiles, TILE_P), np.int64)
    n_full = (n_tiles // CHUNK) * CHUNK
    u = np.arange(n_full)[:, None]
    q = np.arange(TILE_P)[None, :]
    ni[:n_full] = (u // CHUNK) * (CHUNK * TILE_P) + CHUNK * q + u % CHUNK
    rem = n_tiles - n_full
    if rem:
        s = np.arange(rem)[:, None] - 0
        ni[n_full:] = n_full * TILE_P + rem * q + s
    ni_flat = ni.reshape(-1)

    in_maps = []
    for c in range(N_CORES):
        xtp = np.zeros((FB, n_nodes), ml_dtypes.bfloat16)
        xtp[:, :NODES_REAL_PER_CORE] = XT[
            :, c * NODES_REAL_PER_CORE : (c + 1) * NODES_REAL_PER_CORE
        ]
        xc = np.ascontiguousarray(xtp[:, ni_flat])
        in_maps.append({"x": xc, "w1b": w1b, "w2b": w2rows, "b2rep": b2rep})

    trace = os.environ.get("KERNEL_TRACE", "0") == "1"
    kwargs = {}
    if os.environ.get("KERNEL_TRACE_DIR"):
        kwargs["tmpdir"] = os.environ["KERNEL_TRACE_DIR"]
    res = bass_utils.run_bass_kernel_spmd(
        nc, in_maps, core_ids=list(range(N_CORES)), trace=trace, **kwargs
    )
    _CACHE["last_exec_ns"] = res.exec_time_ns
    _CACHE["last_res"] = res
    out = np.concatenate(
        [res.results[c]["y"][:NODES_REAL_PER_CORE] for c in range(N_CORES)],
        axis=0,
    )
    return np.ascontiguousarray(out, dtype=np.float32)



# revision 3
# speedup vs baseline: 1.1709x; 1.1709x over previous
"""GNN message-passing aggregation kernel for Trainium2 (8 NeuronCores).

Math: y[n,o] = mean_k relu(mailbox[n,k,:] @ W1 + b1) @ W2 + b2
  mailbox [500000, 16, 7] fp32, W1 [7,40], W2 [40,3], b1 == 0 (asserted).

Host prep: X^T [112, nodes] bf16 per core (transpose + cast on host), so
the per-tile X^T [112,128] slice is the PE stationary directly -- no
on-device transpose, no dtype cast.

Per 128-node tile, z = X W1blk (640 cols) lands in PSUM via 2 matmuls.
Drain path by tile index mod 3 (PATHS):
  'd'  (i%3==0) DVE fused abs-reduce (sum_k relu z = (sum z + sum |z|)/2;
       the sum-z term goes to yps directly via a 3-col PE matmul with
       tile_k(W1 @ W2)/32 weights -- no ACT copy, no identity rows)
  'g2' (else)   ACT relu-drain (pair-permuted weight cols) to bf16 + one
       GPSIMD halving add + short DVE k-reduce
Per tile-TRIPLE the three 40-col hsums (cols 0:40/40:80/80:120 of a
[128,128] hb buffer) go through one DMA transpose into a [128,128]
stationary, then ONE 9-col matmul applies the zero-masked W2 stack
(W2/32 rows 0:40 for the abs hsum, W2/16 rows 40:80 and 80:120 for the
relu hsums) accumulating y into a GROUP-tile PSUM group; groups flush
via DVE add (+b2) and a chunked DMA out.

A warm-up burst of dummy matmuls at t=0 trips the PE HAM clock gate to
K=8/8 (2.4 GHz); without it the per-tile matmul cadence never sustains
a full 3.4 us busy window and the whole kernel runs at 1.2 GHz.

Sharding: pure data parallel over nodes, 62500/core (padded 62592).
"""

import os
import numpy as np
import ml_dtypes
from contextlib import ExitStack

import concourse.bass as bass
import concourse.bacc as bacc
import concourse.tile as tile
import concourse.mybir as mybir
from concourse.bass import ds, ts
from concourse import bass_utils

F32 = mybir.dt.float32
BF16 = mybir.dt.bfloat16

N_FULL = 500000
K, F_IN, F_HID, F_OUT = 16, 7, 40, 3
FB = K * F_IN              # 112
N_CORES = 8
TILE_P = 128
NODES_REAL_PER_CORE = N_FULL // N_CORES          # 62500
CHUNK = 16                 # node-tiles per input DMA
GROUP = 144                # tiles per y-psum group (144*3 = 432 psum cols);
                           # must divide by 3 (triples) and 16 (CHUNK)

# drain-path pattern over tile index i % 3: slot 0 is 'd' (abs half,
# W2/32 + PE-direct), slots 1,2 are 'g2' (relu, W2/16).
N_PAT = 3

# L2 matmuls are emitted LAG triples behind their transpose so the PE
# FIFO never head-of-line blocks on an in-flight DMA transpose.
LAG = 3

HB_BUFS = 4

# number of PE warm-up matmuls (512 cols each) before the main loop
WARMUP_MM = int(os.environ.get("KERNEL_WARMUP", "14"))

# w1b column layout: [0:640) k-inner z cols (col 16j+k), [640:643) direct
# (W1 @ W2)/32 cols, [643:1283) g2-permuted z cols (8j+k for k<8 at
# +643, 320+8j+k-8 for k>=8).
W1B_COLS = 1283


def build(nc, n_tiles, level=4):
    """Emit the full per-core program into nc.

    level: ablation. 4 = full; 3 = no L2/transpose (y = b2 only);
    2 = L1 + drains, no hb consumers; 1 = L1 matmuls only; 0 = DMA only.
    """
    n_nodes = n_tiles * TILE_P
    x = nc.dram_tensor("x", (FB, n_nodes), BF16, kind="ExternalInput")
    w1b = nc.dram_tensor("w1b", (FB, W1B_COLS), BF16, kind="ExternalInput")
    w2b = nc.dram_tensor("w2b", (TILE_P, 3 * F_OUT), BF16, kind="ExternalInput")
    b2rep = nc.dram_tensor("b2rep", (TILE_P, 3 * GROUP), F32, kind="ExternalInput")
    y = nc.dram_tensor("y", (n_nodes, F_OUT), F32, kind="ExternalOutput")

    xap = x.ap()
    yap = y.ap()

    with ExitStack() as ctx:
        tc = ctx.enter_context(tile.TileContext(nc))
        const = ctx.enter_context(tc.tile_pool(name="const", bufs=1))
        xinp = ctx.enter_context(tc.tile_pool(name="xin", bufs=4))
        zp = ctx.enter_context(tc.tile_pool(name="z", bufs=3, space="PSUM"))
        habsp = ctx.enter_context(tc.tile_pool(name="habs", bufs=5))
        treep = ctx.enter_context(tc.tile_pool(name="tree", bufs=5))
        htp = ctx.enter_context(tc.tile_pool(name="ht", bufs=LAG + 2))
        ypsp = ctx.enter_context(tc.tile_pool(name="yps", bufs=2, space="PSUM"))
        ysbp = ctx.enter_context(tc.tile_pool(name="ysb", bufs=2))

        w1b_sb = const.tile([FB, W1B_COLS], BF16)
        nc.sync.dma_start(w1b_sb[:], w1b.ap())
        w2b_sb = const.tile([TILE_P, 3 * F_OUT], BF16)
        nc.sync.dma_start(w2b_sb[:], w2b.ap())
        b2rep_sb = const.tile([TILE_P, 3 * GROUP], F32, tag="b2rep")
        nc.sync.dma_start(b2rep_sb[:], b2rep.ap())
        zconst = const.tile([FB, 128], BF16)
        nc.gpsimd.memset(zconst[:], 0.0)

        # fixed hb buffers (stable tensor ids): cols 120:128 are zeroed
        # once and never rewritten; they feed zero rows of w2b after
        # transpose but must be finite, not garbage.
        hb_bufs = []
        for bi in range(HB_BUFS):
            hb0 = const.tile([TILE_P, 128], BF16, tag=f"hb{bi}")
            nc.gpsimd.memset(hb0[:, 120:128], 0.0)
            hb_bufs.append(hb0)

        # PE warm-up: back-to-back 512-col matmuls from zconst into the
        # first z psum buffer; no readers, so they retire freely. ~14
        # matmuls ~= 4+ us of sustained PE busy -> HAM K=8/8.
        if WARMUP_MM:
            zw = zp.tile([TILE_P, 640], F32, tag="z")
            for _ in range(WARMUP_MM):
                nc.tensor.matmul(
                    zw[:, 0:512], zconst[:], w1b_sb[:, 0:512],
                    start=True, stop=True,
                )

        yps_by_group = {}
        pending = []

        def pop_pending():
            e = pending.pop(0)
            nc.tensor.matmul(
                e["yps"][:, e["cols"]], e["ht"][:, :], w2b_sb[:, e["wcols"]],
                start=False, stop=False, skip_group_check=True,
            )
            if e["flush"] is not None:
                flush_group(*e["flush"])

        def flush_group(g_yps, g_base, g_ntiles):
            """Drain yps group to SBUF (+b2) and DMA to DRAM."""
            ncols = 3 * g_ntiles
            ysb = ysbp.tile([TILE_P, 3 * GROUP], F32, tag="ysb")
            if g_yps is None:
                nc.vector.tensor_copy(ysb[:, 0:ncols], b2rep_sb[:, 0:ncols])
            else:
                # close the bank's accumulation group (adds zero, full-bank
                # WAW orders it after every per-tile accumulate).
                nc.tensor.matmul(
                    g_yps[:, 0 : 3 * GROUP], zconst[:],
                    w1b_sb[:, 0 : 3 * GROUP],
                    start=False, stop=True, skip_group_check=True,
                )
                nc.vector.tensor_add(
                    ysb[:, 0:ncols], g_yps[:, 0:ncols], b2rep_sb[:, 0:ncols]
                )
            n_full_chunks = g_ntiles // CHUNK
            if n_full_chunks:
                nn = n_full_chunks * CHUNK * TILE_P
                dst = yap[ds(g_base * TILE_P, nn), :].rearrange(
                    "(c q s) o -> q c s o", q=TILE_P, s=CHUNK
                )
                src_ap = ysb[:, 0 : n_full_chunks * CHUNK * 3].rearrange(
                    "q (c s o) -> q c s o", s=CHUNK, o=3
                )
                nc.sync.dma_start(dst, src_ap)
            rem = g_ntiles - n_full_chunks * CHUNK
            if rem:
                base = g_base + n_full_chunks * CHUNK
                dst = yap[ds(base * TILE_P, rem * TILE_P), :].rearrange(
                    "(q s) o -> q s o", s=rem
                )
                src_ap = ysb[
                    :, n_full_chunks * CHUNK * 3 : g_ntiles * 3
                ].rearrange("q (s o) -> q s o", o=3)
                nc.sync.dma_start(dst, src_ap)

        # Software-pipelined emission: per-engine queues are strict
        # program order, so each tile's dependent stages are emitted N
        # rounds later to keep every queue head runnable.
        zabs, habss, trs = {}, {}, {}
        xins = {}
        n_chunks = (n_tiles + CHUNK - 1) // CHUNK
        PREFETCH = 2

        def emit_chunk_dma(c):
            if c >= n_chunks or c in xins:
                return
            nch = min(CHUNK, n_tiles - c * CHUNK)
            xin = xinp.tile([FB, CHUNK * TILE_P], BF16, tag="xin")
            # scalar HWDGE queue: keeps input DMAs from queueing behind
            # hsum transposes on sync.
            nc.scalar.dma_start(
                xin[:, 0 : nch * TILE_P],
                xap[:, ds(c * CHUNK * TILE_P, nch * TILE_P)],
            )
            xins[c] = xin

        def stage_front(i):
            c, s = divmod(i, CHUNK)
            if i == 0:
                for pc in range(PREFETCH + 1):
                    emit_chunk_dma(pc)
            elif s == 0:
                emit_chunk_dma(c + PREFETCH)
                xins.pop(c - 2, None)
            g_idx = i % GROUP
            if g_idx == 0 and level >= 4:
                yps = ypsp.tile([TILE_P, 3 * GROUP], F32, tag="yps")
                yps_by_group[i // GROUP] = yps
                # open the bank's single accumulation group (whole region
                # pending-zero; orders before every per-tile accumulate).
                nc.tensor.matmul(
                    yps[:, 0 : 3 * GROUP], zconst[:],
                    w1b_sb[:, 0 : 3 * GROUP],
                    start=True, stop=False, skip_group_check=True,
                )
            if level < 1:
                return
            path = 'd' if i % N_PAT == 0 else 'g2'
            wofs = 643 if path == 'g2' else 0
            xts = xins[c][:, ds(s * TILE_P, TILE_P)]
            zab = zp.tile([TILE_P, 640], F32, tag="z")
            zabs[i] = zab
            nc.tensor.matmul(
                zab[:, 0:512], xts, w1b_sb[:, wofs : wofs + 512],
                start=True, stop=True,
            )
            nc.tensor.matmul(
                zab[:, 512:640],
                xts, w1b_sb[:, wofs + 512 : wofs + 640],
                start=True, stop=True,
            )
            if path == 'd' and level >= 4:
                # direct term (sum_k z) @ W2/32 = X @ tile_k(W1@W2)/32,
                # accumulated straight into the y psum group by the PE.
                nc.tensor.matmul(
                    yps_by_group[i // GROUP][:, ts(g_idx, 3)],
                    xts, w1b_sb[:, 640:643],
                    start=False, stop=False, skip_group_check=True,
                )

        def stage_drain1(i):
            if level < 2:
                zabs.pop(i, None)
                return
            path = 'd' if i % N_PAT == 0 else 'g2'
            zab = zabs.pop(i)
            hb = hb_bufs[(i // 3) % HB_BUFS]
            hcol = 40 * (i % 3)
            with nc.allow_low_precision("bf16 hsum is within tolerance"):
                if path == 'd':
                    nc.vector.tensor_reduce(
                        hb[:, hcol : hcol + 40],
                        zab[:, 0:640].rearrange("q (j k) -> q j k", k=K),
                        axis=mybir.AxisListType.X,
                        op=mybir.AluOpType.add,
                        apply_absolute_value=True,
                    )
                else:
                    habs = habsp.tile([TILE_P, 640], BF16, tag="habs")
                    habss[i] = habs
                    nc.scalar.activation(
                        habs[:], zab[:, 0:640],
                        mybir.ActivationFunctionType.Relu,
                    )

        def stage_fold(i):
            if level < 2 or i % N_PAT == 0:
                return
            habs = habss.pop(i)
            tr = treep.tile([TILE_P, 320], BF16, tag="tree")
            trs[i] = tr
            nc.gpsimd.tensor_add(
                tr[:, 0:320], habs[:, 0:320], habs[:, 320:640]
            )

        def stage_red2(i):
            if level < 2 or i % N_PAT == 0:
                return
            hb = hb_bufs[(i // 3) % HB_BUFS]
            hcol = 40 * (i % 3)
            tr = trs.pop(i)
            with nc.allow_low_precision("bf16 hsum is within tolerance"):
                nc.vector.tensor_reduce(
                    hb[:, hcol : hcol + 40],
                    tr[:, 0:320].rearrange("q (j k) -> q j k", k=8),
                    axis=mybir.AxisListType.X,
                    op=mybir.AluOpType.add,
                )

        def stage_xpose(i):
            if level < 4 or i % 3 != 2:
                return
            g_idx = i % GROUP
            hb = hb_bufs[(i // 3) % HB_BUFS]
            ht = htp.tile([128, 128], BF16, tag="ht")
            nc.sync.dma_start(ht[:], hb[:], transpose=True)
            # one 9-col L2 matmul per triple, emitted LAG triples later:
            # ht rows 0:40 (W2/32, tile i-2) -> cols 3(g-2):3(g-2)+3,
            # rows 40:80 (W2/16, tile i-1) -> next 3, rows 80:120 (W2/16,
            # tile i) -> next 3.
            e = {
                "ht": ht,
                "yps": yps_by_group[i // GROUP],
                "cols": ds(3 * (g_idx - 2), 9),
                "wcols": ds(0, 9),
                "flush": None,
            }
            if g_idx == GROUP - 1 or i == n_tiles - 1:
                e["flush"] = (yps_by_group[i // GROUP], i - g_idx, g_idx + 1)
            pending.append(e)
            if len(pending) > LAG:
                pop_pending()

        for r in range(n_tiles + 5):
            if r < n_tiles:
                stage_front(r)
            if 0 <= r - 1 < n_tiles:
                stage_drain1(r - 1)
            if 0 <= r - 2 < n_tiles:
                stage_fold(r - 2)
            if 0 <= r - 3 < n_tiles:
                stage_red2(r - 3)
            if 0 <= r - 4 < n_tiles:
                stage_xpose(r - 4)

        while pending:
            pop_pending()


_CACHE = {}


def _get_prog():
    key = "prog"
    if key not in _CACHE:
        nc = bacc.Bacc(
            "TRN2", target_bir_lowering=False, debug=False,
            num_devices=N_CORES,
        )
        n_tiles = (NODES_REAL_PER_CORE + TILE_P - 1) // TILE_P  # 489
        build(nc, n_tiles, level=int(os.environ.get("KERNEL_LEVEL", "4")))
        nc.finalize()
        _CACHE[key] = (nc, n_tiles)
    return _CACHE[key]


def _host_weights(W1, b1, W2, b2):
    W1 = np.asarray(W1, np.float32)
    W2 = np.asarray(W2, np.float32)
    b2 = np.asarray(b2, np.float32)

    # k-inner z cols: col 16*j + k
    w1ki = np.zeros((K, F_IN, F_HID, K), np.float32)
    for k in range(K):
        w1ki[k, :, :, k] = W1
    w1ki = w1ki.reshape(FB, F_HID * K)
    # direct term: sum_k z_k @ W2/32 = X @ tile_k(W1 @ W2)/32
    wdir = np.tile(W1 @ W2 / 32.0, (K, 1))  # [112, 3]
    # g2-permuted cols: halves foldable by one contiguous add, result
    # j-major k-inner(8): col 8j+k for k<8, col 320+8j+(k-8) for k>=8.
    w1g2 = np.zeros((FB, 640), np.float32)
    for k in range(K):
        for j in range(F_HID):
            col = 8 * j + k if k < 8 else 320 + 8 * j + (k - 8)
            w1g2[7 * k : 7 * k + 7, col] = W1[:, j]
    w1b = np.concatenate([w1ki, wdir, w1g2], axis=1).astype(ml_dtypes.bfloat16)

    w2rows = np.zeros((TILE_P, 3 * F_OUT), np.float32)
    w2rows[0:F_HID, 0:F_OUT] = W2 / 32.0                      # abs hsum
    w2rows[40 : 40 + F_HID, F_OUT : 2 * F_OUT] = W2 / 16.0    # relu hsum B
    w2rows[80 : 80 + F_HID, 2 * F_OUT : 3 * F_OUT] = W2 / 16.0  # relu hsum C
    w2rows = w2rows.astype(ml_dtypes.bfloat16)
    b2rep = np.tile(b2, (TILE_P, GROUP)).astype(np.float32)
    return w1b, w2rows, b2rep


def kernel(mailbox, W1, b1, W2, b2, **_unused):
    mailbox = np.asarray(mailbox)
    assert mailbox.shape == (N_FULL, K, F_IN), mailbox.shape
    b1 = np.asarray(b1, np.float32)
    assert np.abs(b1).max() == 0.0, "kernel assumes b1 == 0"

    nc, n_tiles = _get_prog()
    n_nodes = n_tiles * TILE_P

    X = np.ascontiguousarray(mailbox, dtype=np.float32).reshape(N_FULL, FB)
    XT = np.ascontiguousarray(X.T.astype(ml_dtypes.bfloat16))  # [112, N]
    w1b, w2rows, b2rep = _host_weights(W1, b1, W2, np.asarray(b2, np.float32))

    # node-interleaved tiling: within each CHUNK-tile block, node
    # base + CHUNK*q + s sits at (tile s, partition q), so the output
    # DMA writes CHUNK*3-element contiguous runs per partition.
    ni = np.empty((n_tiles, TILE_P), np.int64)
    n_full = (n_tiles // CHUNK) * CHUNK
    u = np.arange(n_full)[:, None]
    q = np.arange(TILE_P)[None, :]
    ni[:n_full] = (u // CHUNK) * (CHUNK * TILE_P) + CHUNK * q + u % CHUNK
    rem = n_tiles - n_full
    if rem:
        s = np.arange(rem)[:, None] - 0
        ni[n_full:] = n_full * TILE_P + rem * q + s
    ni_flat = ni.reshape(-1)

    in_maps = []
    for c in range(N_CORES):
        xtp = np.zeros((FB, n_nodes), ml_dtypes.bfloat16)
        xtp[:, :NODES_REAL_PER_CORE] = XT[
            :, c * NODES_REAL_PER_CORE : (c + 1) * NODES_REAL_PER_CORE
        ]
        xc = np.ascontiguousarray(xtp[:, ni_flat])
        in_maps.append({"x": xc, "w1b": w1b, "w2b": w2rows, "b2rep": b2rep})

    trace = os.environ.get("KERNEL_TRACE", "0") == "1"
    kwargs = {}
    if os.environ.get("KERNEL_TRACE_DIR"):
        kwargs["tmpdir"] = os.environ["KERNEL_TRACE_DIR"]
    res = bass_utils.run_bass_kernel_spmd(
        nc, in_maps, core_ids=list(range(N_CORES)), trace=trace, **kwargs
    )
    _CACHE["last_exec_ns"] = res.exec_time_ns
    _CACHE["last_res"] = res
    out = np.concatenate(
        [res.results[c]["y"][:NODES_REAL_PER_CORE] for c in range(N_CORES)],
        axis=0,
    )
    return np.ascontiguousarray(out, dtype=np.float32)


# revision 7
# speedup vs baseline: 1.2035x; 1.0278x over previous
"""GNN message-passing aggregation kernel for Trainium2 (8 NeuronCores).

Math: y[n,o] = mean_k relu(mailbox[n,k,:] @ W1 + b1) @ W2 + b2
  mailbox [500000, 16, 7] fp32, W1 [7,40], W2 [40,3], b1 == 0 (asserted).

Host prep: X^T [112, nodes] bf16 per core (transpose + cast on host), so
the per-tile X^T [112,128] slice is the PE stationary directly -- no
on-device transpose, no dtype cast.

Per 128-node tile, z = X W1blk (640 cols) lands in PSUM via 2 matmuls.
Drain path by tile index mod 3 (PATHS):
  'd'  (i%3==0) DVE fused abs-reduce (sum_k relu z = (sum z + sum |z|)/2;
       the sum-z term goes to yps directly via a 3-col PE matmul with
       tile_k(W1 @ W2)/32 weights -- no ACT copy, no identity rows)
  'g2' (else)   ACT relu-drain (pair-permuted weight cols) to bf16 + one
       GPSIMD halving add + short DVE k-reduce
Per tile-TRIPLE the three 40-col hsums (cols 0:40/40:80/80:120 of a
[128,128] hb buffer) go through one DMA transpose into a [128,128]
stationary, then ONE 9-col matmul applies the zero-masked W2 stack
(W2/32 rows 0:40 for the abs hsum, W2/16 rows 40:80 and 80:120 for the
relu hsums) accumulating y into a GROUP-tile PSUM group; groups flush
via DVE add (+b2) and a chunked DMA out.

A warm-up burst of dummy matmuls at t=0 trips the PE HAM clock gate to
K=8/8 (2.4 GHz); without it the per-tile matmul cadence never sustains
a full 3.4 us busy window and the whole kernel runs at 1.2 GHz.

Sharding: pure data parallel over nodes, 62500/core (padded 62592).
"""

import os
import numpy as np
import ml_dtypes
from contextlib import ExitStack

import concourse.bass as bass
import concourse.bacc as bacc
import concourse.tile as tile
import concourse.mybir as mybir
from concourse.bass import ds, ts
from concourse import bass_utils

F32 = mybir.dt.float32
BF16 = mybir.dt.bfloat16

N_FULL = 500000
K, F_IN, F_HID, F_OUT = 16, 7, 40, 3
FB = K * F_IN              # 112
N_CORES = 8
TILE_P = 128
NODES_REAL_PER_CORE = N_FULL // N_CORES          # 62500
CHUNK = 16                 # node-tiles per input DMA
GROUP = 144                # tiles per y-psum group (144*3 = 432 psum cols);
                           # must divide by 3 (triples) and 16 (CHUNK)

# drain-path pattern over tile index i % 3: slot 0 is 'd' (abs half,
# W2/32 + PE-direct), slots 1,2 are 'g2' (relu, W2/16).
N_PAT = 3

# L2 matmuls are emitted LAG triples behind their transpose so the PE
# FIFO never head-of-line blocks on an in-flight DMA transpose.
LAG = int(os.environ.get("KERNEL_LAG", "9"))

HB_BUFS = 6

# number of PE warm-up matmuls (512 cols each) before the main loop
WARMUP_MM = int(os.environ.get("KERNEL_WARMUP", "14"))

# w1b column layout: [0:640) k-inner z cols (col 16j+k), [640:643) direct
# (W1 @ W2)/32 cols, [643:1283) g2-permuted z cols (8j+k for k<8 at
# +643, 320+8j+k-8 for k>=8).
W1B_COLS = 1283


def build(nc, n_tiles, level=4):
    """Emit the full per-core program into nc.

    level: ablation. 4 = full; 3 = no L2/transpose (y = b2 only);
    2 = L1 + drains, no hb consumers; 1 = L1 matmuls only; 0 = DMA only.
    """
    n_nodes = n_tiles * TILE_P
    x = nc.dram_tensor("x", (FB, n_nodes), BF16, kind="ExternalInput")
    w1b = nc.dram_tensor("w1b", (FB, W1B_COLS), BF16, kind="ExternalInput")
    w2b = nc.dram_tensor("w2b", (TILE_P, 3 * F_OUT), BF16, kind="ExternalInput")
    b2rep = nc.dram_tensor("b2rep", (TILE_P, 3 * GROUP), F32, kind="ExternalInput")
    y = nc.dram_tensor("y", (n_nodes, F_OUT), F32, kind="ExternalOutput")

    xap = x.ap()
    yap = y.ap()

    with ExitStack() as ctx:
        tc = ctx.enter_context(tile.TileContext(nc))
        const = ctx.enter_context(tc.tile_pool(name="const", bufs=1))
        xinp = ctx.enter_context(tc.tile_pool(name="xin", bufs=4))
        zp = ctx.enter_context(tc.tile_pool(name="z", bufs=3, space="PSUM"))
        habsp = ctx.enter_context(tc.tile_pool(name="habs", bufs=6))
        treep = ctx.enter_context(tc.tile_pool(name="tree", bufs=6))
        htp = ctx.enter_context(tc.tile_pool(name="ht", bufs=LAG + 2))
        ypsp = ctx.enter_context(tc.tile_pool(name="yps", bufs=2, space="PSUM"))
        ysbp = ctx.enter_context(tc.tile_pool(name="ysb", bufs=2))

        w1b_sb = const.tile([FB, W1B_COLS], BF16)
        nc.sync.dma_start(w1b_sb[:], w1b.ap())
        w2b_sb = const.tile([TILE_P, 3 * F_OUT], BF16)
        nc.sync.dma_start(w2b_sb[:], w2b.ap())
        b2rep_sb = const.tile([TILE_P, 3 * GROUP], F32, tag="b2rep")
        nc.sync.dma_start(b2rep_sb[:], b2rep.ap())
        zconst = const.tile([FB, 128], BF16)
        nc.gpsimd.memset(zconst[:], 0.0)

        # fixed hb buffers (stable tensor ids): cols 120:128 are zeroed
        # once and never rewritten; they feed zero rows of w2b after
        # transpose but must be finite, not garbage.
        hb_bufs = []
        for bi in range(HB_BUFS):
            hb0 = const.tile([TILE_P, 128], BF16, tag=f"hb{bi}")
            nc.gpsimd.memset(hb0[:, 120:128], 0.0)
            hb_bufs.append(hb0)

        # PE warm-up: back-to-back 512-col matmuls from zconst into the
        # first z psum buffer; no readers, so they retire freely. ~14
        # matmuls ~= 4+ us of sustained PE busy -> HAM K=8/8.
        if WARMUP_MM:
            zw = zp.tile([TILE_P, 640], F32, tag="z")
            for _ in range(WARMUP_MM):
                nc.tensor.matmul(
                    zw[:, 0:512], zconst[:], w1b_sb[:, 0:512],
                    start=True, stop=True,
                )

        yps_by_group = {}
        pending = []

        def pop_pending():
            e = pending.pop(0)
            nc.tensor.matmul(
                e["yps"][:, e["cols"]], e["ht"][:, :], w2b_sb[:, e["wcols"]],
                start=False, stop=False, skip_group_check=True,
            )
            if e["flush"] is not None:
                flush_group(*e["flush"])

        def flush_group(g_yps, g_base, g_ntiles):
            """Drain yps group to SBUF (+b2) and DMA to DRAM."""
            ncols = 3 * g_ntiles
            ysb = ysbp.tile([TILE_P, 3 * GROUP], F32, tag="ysb")
            if g_yps is None:
                nc.vector.tensor_copy(ysb[:, 0:ncols], b2rep_sb[:, 0:ncols])
            else:
                # close the bank's accumulation group (adds zero, full-bank
                # WAW orders it after every per-tile accumulate).
                nc.tensor.matmul(
                    g_yps[:, 0 : 3 * GROUP], zconst[:],
                    w1b_sb[:, 0 : 3 * GROUP],
                    start=False, stop=True, skip_group_check=True,
                )
                nc.vector.tensor_add(
                    ysb[:, 0:ncols], g_yps[:, 0:ncols], b2rep_sb[:, 0:ncols]
                )
            n_full_chunks = g_ntiles // CHUNK
            if n_full_chunks:
                nn = n_full_chunks * CHUNK * TILE_P
                dst = yap[ds(g_base * TILE_P, nn), :].rearrange(
                    "(c q s) o -> q c s o", q=TILE_P, s=CHUNK
                )
                src_ap = ysb[:, 0 : n_full_chunks * CHUNK * 3].rearrange(
                    "q (c s o) -> q c s o", s=CHUNK, o=3
                )
                nc.sync.dma_start(dst, src_ap)
            rem = g_ntiles - n_full_chunks * CHUNK
            if rem:
                base = g_base + n_full_chunks * CHUNK
                dst = yap[ds(base * TILE_P, rem * TILE_P), :].rearrange(
                    "(q s) o -> q s o", s=rem
                )
                src_ap = ysb[
                    :, n_full_chunks * CHUNK * 3 : g_ntiles * 3
                ].rearrange("q (s o) -> q s o", o=3)
                nc.sync.dma_start(dst, src_ap)

        # Software-pipelined emission: per-engine queues are strict
        # program order, so each tile's dependent stages are emitted N
        # rounds later to keep every queue head runnable.
        zabs, habss, trs = {}, {}, {}
        xins = {}
        n_chunks = (n_tiles + CHUNK - 1) // CHUNK
        PREFETCH = 2

        def emit_chunk_dma(c):
            if c >= n_chunks or c in xins:
                return
            nch = min(CHUNK, n_tiles - c * CHUNK)
            xin = xinp.tile([FB, CHUNK * TILE_P], BF16, tag="xin")
            # sync HWDGE queue: keeps the input-DMA issue cost off the
            # ACT queue, which is saturated with relu drains.
            nc.sync.dma_start(
                xin[:, 0 : nch * TILE_P],
                xap[:, ds(c * CHUNK * TILE_P, nch * TILE_P)],
            )
            xins[c] = xin

        def stage_front(i):
            c, s = divmod(i, CHUNK)
            if i == 0:
                for pc in range(PREFETCH + 1):
                    emit_chunk_dma(pc)
            elif s == 0:
                emit_chunk_dma(c + PREFETCH)
                xins.pop(c - 2, None)
            g_idx = i % GROUP
            if g_idx == 0 and level >= 4:
                yps = ypsp.tile([TILE_P, 3 * GROUP], F32, tag="yps")
                yps_by_group[i // GROUP] = yps
                # open the bank's single accumulation group (whole region
                # pending-zero; orders before every per-tile accumulate).
                nc.tensor.matmul(
                    yps[:, 0 : 3 * GROUP], zconst[:],
                    w1b_sb[:, 0 : 3 * GROUP],
                    start=True, stop=False, skip_group_check=True,
                )
            if level < 1:
                return
            path = 'd' if i % N_PAT == 0 else 'g2'
            wofs = 643 if path == 'g2' else 0
            xts = xins[c][:, ds(s * TILE_P, TILE_P)]
            zab = zp.tile([TILE_P, 640], F32, tag="z")
            zabs[i] = zab
            nc.tensor.matmul(
                zab[:, 0:512], xts, w1b_sb[:, wofs : wofs + 512],
                start=True, stop=True,
            )
            nc.tensor.matmul(
                zab[:, 512:640],
                xts, w1b_sb[:, wofs + 512 : wofs + 640],
                start=True, stop=True,
            )
            if path == 'd' and level >= 4:
                # direct term (sum_k z) @ W2/32 = X @ tile_k(W1@W2)/32,
                # accumulated straight into the y psum group by the PE.
                nc.tensor.matmul(
                    yps_by_group[i // GROUP][:, ts(g_idx, 3)],
                    xts, w1b_sb[:, 640:643],
                    start=False, stop=False, skip_group_check=True,
                )

        def stage_drain1(i):
            if level < 2:
                zabs.pop(i, None)
                return
            path = 'd' if i % N_PAT == 0 else 'g2'
            zab = zabs.pop(i)
            hb = hb_bufs[(i // 3) % HB_BUFS]
            hcol = 40 * (i % 3)
            with nc.allow_low_precision("bf16 hsum is within tolerance"):
                if path == 'd':
                    nc.vector.tensor_reduce(
                        hb[:, hcol : hcol + 40],
                        zab[:, 0:640].rearrange("q (j k) -> q j k", k=K),
                        axis=mybir.AxisListType.X,
                        op=mybir.AluOpType.add,
                        apply_absolute_value=True,
                    )
                else:
                    habs = habsp.tile([TILE_P, 640], BF16, tag="habs")
                    habss[i] = habs
                    nc.scalar.activation(
                        habs[:], zab[:, 0:640],
                        mybir.ActivationFunctionType.Relu,
                    )

        def stage_fold(i):
            if level < 2 or i % N_PAT == 0:
                return
            habs = habss.pop(i)
            tr = treep.tile([TILE_P, 320], BF16, tag="tree")
            trs[i] = tr
            nc.gpsimd.tensor_add(
                tr[:, 0:320], habs[:, 0:320], habs[:, 320:640]
            )

        def stage_red2(i):
            if level < 2 or i % N_PAT == 0:
                return
            hb = hb_bufs[(i // 3) % HB_BUFS]
            hcol = 40 * (i % 3)
            tr = trs.pop(i)
            with nc.allow_low_precision("bf16 hsum is within tolerance"):
                nc.vector.tensor_reduce(
                    hb[:, hcol : hcol + 40],
                    tr[:, 0:320].rearrange("q (j k) -> q j k", k=8),
                    axis=mybir.AxisListType.X,
                    op=mybir.AluOpType.add,
                )

        def stage_xpose(i):
            if level < 4 or i % 3 != 2:
                return
            g_idx = i % GROUP
            hb = hb_bufs[(i // 3) % HB_BUFS]
            ht = htp.tile([128, 128], BF16, tag="ht")
            nc.sync.dma_start(ht[:], hb[:], transpose=True)
            # one 9-col L2 matmul per triple, emitted LAG triples later:
            # ht rows 0:40 (W2/32, tile i-2) -> cols 3(g-2):3(g-2)+3,
            # rows 40:80 (W2/16, tile i-1) -> next 3, rows 80:120 (W2/16,
            # tile i) -> next 3.
            e = {
                "ht": ht,
                "yps": yps_by_group[i // GROUP],
                "cols": ds(3 * (g_idx - 2), 9),
                "wcols": ds(0, 9),
                "flush": None,
            }
            if g_idx == GROUP - 1 or i == n_tiles - 1:
                e["flush"] = (yps_by_group[i // GROUP], i - g_idx, g_idx + 1)
            pending.append(e)
            if len(pending) > LAG:
                pop_pending()

        for r in range(n_tiles + 5):
            if r < n_tiles:
                stage_front(r)
            if 0 <= r - 1 < n_tiles:
                stage_drain1(r - 1)
            if 0 <= r - 2 < n_tiles:
                stage_fold(r - 2)
            if 0 <= r - 3 < n_tiles:
                stage_red2(r - 3)
            if 0 <= r - 4 < n_tiles:
                stage_xpose(r - 4)

        while pending:
            pop_pending()


_CACHE = {}


def _get_prog():
    key = "prog"
    if key not in _CACHE:
        nc = bacc.Bacc(
            "TRN2", target_bir_lowering=False, debug=False,
            num_devices=N_CORES,
        )
        n_tiles = (NODES_REAL_PER_CORE + TILE_P - 1) // TILE_P  # 489
        build(nc, n_tiles, level=int(os.environ.get("KERNEL_LEVEL", "4")))
        nc.finalize()
        _CACHE[key] = (nc, n_tiles)
    return _CACHE[key]


def _host_weights(W1, b1, W2, b2):
    W1 = np.asarray(W1, np.float32)
    W2 = np.asarray(W2, np.float32)
    b2 = np.asarray(b2, np.float32)

    # k-inner z cols: col 16*j + k
    w1ki = np.zeros((K, F_IN, F_HID, K), np.float32)
    for k in range(K):
        w1ki[k, :, :, k] = W1
    w1ki = w1ki.reshape(FB, F_HID * K)
    # direct term: sum_k z_k @ W2/32 = X @ tile_k(W1 @ W2)/32
    wdir = np.tile(W1 @ W2 / 32.0, (K, 1))  # [112, 3]
    # g2-permuted cols: halves foldable by one contiguous add, result
    # j-major k-inner(8): col 8j+k for k<8, col 320+8j+(k-8) for k>=8.
    w1g2 = np.zeros((FB, 640), np.float32)
    for k in range(K):
        for j in range(F_HID):
            col = 8 * j + k if k < 8 else 320 + 8 * j + (k - 8)
            w1g2[7 * k : 7 * k + 7, col] = W1[:, j]
    w1b = np.concatenate([w1ki, wdir, w1g2], axis=1).astype(ml_dtypes.bfloat16)

    w2rows = np.zeros((TILE_P, 3 * F_OUT), np.float32)
    w2rows[0:F_HID, 0:F_OUT] = W2 / 32.0                      # abs hsum
    w2rows[40 : 40 + F_HID, F_OUT : 2 * F_OUT] = W2 / 16.0    # relu hsum B
    w2rows[80 : 80 + F_HID, 2 * F_OUT : 3 * F_OUT] = W2 / 16.0  # relu hsum C
    w2rows = w2rows.astype(ml_dtypes.bfloat16)
    b2rep = np.tile(b2, (TILE_P, GROUP)).astype(np.float32)
    return w1b, w2rows, b2rep


def kernel(mailbox, W1, b1, W2, b2, **_unused):
    mailbox = np.asarray(mailbox)
    assert mailbox.shape == (N_FULL, K, F_IN), mailbox.shape
    b1 = np.asarray(b1, np.float32)
    assert np.abs(b1).max() == 0.0, "kernel assumes b1 == 0"

    nc, n_tiles = _get_prog()
    n_nodes = n_tiles * TILE_P

    X = np.ascontiguousarray(mailbox, dtype=np.float32).reshape(N_FULL, FB)
    XT = np.ascontiguousarray(X.T.astype(ml_dtypes.bfloat16))  # [112, N]
    w1b, w2rows, b2rep = _host_weights(W1, b1, W2, np.asarray(b2, np.float32))

    # node-interleaved tiling: within each CHUNK-tile block, node
    # base + CHUNK*q + s sits at (tile s, partition q), so the output
    # DMA writes CHUNK*3-element contiguous runs per partition.
    ni = np.empty((n_tiles, TILE_P), np.int64)
    n_full = (n_tiles // CHUNK) * CHUNK
    u = np.arange(n_full)[:, None]
    q = np.arange(TILE_P)[None, :]
    ni[:n_full] = (u // CHUNK) * (CHUNK * TILE_P) + CHUNK * q + u % CHUNK
    rem = n_tiles - n_full
    if rem:
        s = np.arange(rem)[:, None] - 0
        ni[n_full:] = n_full * TILE_P + rem * q + s
    ni_flat = ni.reshape(-1)

    in_maps = []
    for c in range(N_CORES):
        xtp = np.zeros((FB, n_nodes), ml_dtypes.bfloat16)
        xtp[:, :NODES_REAL_PER_CORE] = XT[
            :, c * NODES_REAL_PER_CORE : (c + 1) * NODES_REAL_PER_CORE
        ]
        xc = np.ascontiguousarray(xtp[:, ni_flat])
        in_maps.append({"x": xc, "w1b": w1b, "w2b": w2rows, "b2rep": b2rep})

    trace = os.environ.get("KERNEL_TRACE", "0") == "1"
    kwargs = {}
    if os.environ.get("KERNEL_TRACE_DIR"):
        kwargs["tmpdir"] = os.environ["KERNEL_TRACE_DIR"]
    res = bass_utils.run_bass_kernel_spmd(
        nc, in_maps, core_ids=list(range(N_CORES)), trace=trace, **kwargs
    )
    _CACHE["last_exec_ns"] = res.exec_time_ns
    _CACHE["last_res"] = res
    out = np.concatenate(
        [res.results[c]["y"][:NODES_REAL_PER_CORE] for c in range(N_CORES)],
        axis=0,
    )
    return np.ascontiguousarray(out, dtype=np.float32)
